# revision 46
# baseline (speedup 1.0000x reference)
"""Mamba block (dense_transformer nn_Block) on 8 Trainium2 NeuronCores.

v3: DVE-critical-path schedule. The selective scan is the hard floor
(~141us of DVE scan + ~85us of DVE elementwise per core); everything
else (PE projections/MLP, Scalar activations, collectives, DMA) is
arranged to hide under it:
- d-interleaved scan groups share one B/C broadcast DMA per (b, group).
- Front is pipelined per batch-half with the AllReduce split into two
  column chunks; dt/z/conv for b1 overlap b0's scans.
- Scalar ops are emission-ordered to batch activation-table sets
  (exp/ln vs silu vs gelu), and LN1 squares run on Scalar, not DVE.
- Backend(b0) (out_proj, LN2, MLP) is injected into b1's scan window;
  per-d AllToAll launches as soon as that d's y3 is ready.
"""
import os
import numpy as np
import ml_dtypes

import concourse.bass as bass
import concourse.bacc as bacc
import concourse.mybir as mybir
import concourse.tile as tile
from contextlib import ExitStack
from concourse.bass_utils import run_bass_kernel_spmd

BF16 = mybir.dt.bfloat16
FP8 = mybir.dt.float8e4
F32 = mybir.dt.float32
AF = mybir.ActivationFunctionType
OP = mybir.AluOpType
bf = ml_dtypes.bfloat16

B, L, E = 2, 1024, 1024
DIN, NST, RDT, KC = 2 * E, 16, 64, 4
EPS = 1e-5
NC = 8
DL = DIN // NC          # 256 channels per core
TOK = B * L             # 2048
TOKB = L // NC          # 128 tokens per core per batch half
TOKC = 2 * TOKB         # 256 tokens per core total
HID = 4 * E             # 4096

_BUILD_CACHE = {}


def _rep0(src_ap, parts=128):
    """Partition-broadcast: prepend a [0, parts] dim to an AP's pattern."""
    return bass.AP(src_ap.tensor, src_ap.offset,
                   [[0, parts]] + [list(p) for p in src_ap.ap])


def _build(ln1b_nonzero):
    key = (ln1b_nonzero,)
    if key in _BUILD_CACHE:
        return _BUILD_CACHE[key]

    nc = bacc.Bacc("TRN2", target_bir_lowering=False, debug=False, num_devices=NC)

    def din(name, shape, dt=BF16):
        return nc.dram_tensor(name, shape, dt, kind="ExternalInput").ap()

    xT = din("xT", [E, TOK])
    win = din("win", [128, 8 * 512])
    sw_in = din("sw_in", [1, 512])
    sb_in = din("sb_in", [128, 4], F32)
    convw = din("convw", [128, 2 * KC], F32)
    convb = din("convb", [128, 2], F32)
    xpw = din("xpw", [128, 2 * 96])
    dtw = din("dtw", [64, 256])
    dtb = din("dtb", [128, 2], F32)
    a_sc = din("a_sc", [128, 2 * NST], F32)
    dvec = din("dvec", [128, 2], F32)
    wo = din("wo", [128, 16 * 8 * 128])
    xresT = din("xresT", [TOKC, E], F32)
    wfc = din("wfc", [128, 8 * HID])
    sbfc = din("sbfc", [1, HID])
    wpj = din("wpj", [128, 32 * E])
    pjb = din("pjb", [1, E])
    ones128 = din("ones128", [128, 1])
    ident = din("ident", [128, 128])

    outT = nc.dram_tensor("outT", [TOKC, E], F32, kind="ExternalOutput").ap()

    cc_dummy_in = nc.dram_tensor("cc_dummy_in", [1, 16], F32)
    cc_dummy_out = nc.dram_tensor("cc_dummy_out", [1, 16], F32, addr_space="Shared")
    ar_ins = [nc.dram_tensor(f"ar_in{b}", [96, L], BF16) for b in range(2)]
    ar_outs = [nc.dram_tensor(f"ar_out{b}", [96, L], BF16, addr_space="Shared")
               for b in range(2)]
    bc_bfs = [nc.dram_tensor(f"bc_bf{b}", [32, L], BF16) for b in range(2)]
    a2a_ins = [[nc.dram_tensor(f"a2a_in{d}{b}", [NC, 128 * TOKB], BF16)
                for b in range(2)] for d in range(2)]
    a2a_outs = [[nc.dram_tensor(f"a2a_out{d}{b}", [NC, 128 * TOKB], BF16)
                 for b in range(2)] for d in range(2)]
    RG = [list(range(NC))]

    with tile.TileContext(nc) as tc, ExitStack() as _stk:
        # warm the collective stream early (absorbs ~40us start delay)
        nc.gpsimd.collective_compute("AllReduce", OP.add, ins=[cc_dummy_in[:]],
                                     outs=[cc_dummy_out[:]], replica_groups=RG)

        cpool = _stk.enter_context(tc.tile_pool(name="consts", bufs=1))
        ones_t = cpool.tile([128, 1], BF16, tag="ones")
        nc.sync.dma_start(ones_t[:], ones128[:])
        ident_t = cpool.tile([128, 128], BF16, tag="ident")
        ones_row = cpool.tile([1, 128], BF16, tag="onesrow")
        nc.sync.dma_start(ones_row[:], ones128[:].rearrange("p q -> q p"))
        ones_row_f = cpool.tile([1, 128], F32, tag="onesrowf")
        nc.vector.tensor_copy(ones_row_f[:], ones_row[:])
        sw_t = cpool.tile([1, 512], BF16, tag="sw")
        convw_t = cpool.tile([128, 2 * KC], F32, tag="convw")
        convb_t = cpool.tile([128, 2], F32, tag="convb")
        xpw_t = cpool.tile([128, 2 * 96], BF16, tag="xpw")
        dtw_t = cpool.tile([64, 256], BF16, tag="dtw")
        dtb_t = cpool.tile([128, 2], F32, tag="dtb")
        asc_t = cpool.tile([128, 2 * NST], F32, tag="asc")
        dvec_t = cpool.tile([128, 2], F32, tag="dvec")
        sbfc_sb = cpool.tile([1, HID], BF16, tag="sbfcr")
        pjb_sb = cpool.tile([1, E], BF16, tag="pjbr")
        eps_t = cpool.tile([128, 1], F32, tag="eps")
        nc.vector.memset(eps_t[:], EPS)
        sbin_t = cpool.tile([128, 4], F32, tag="sbin")
        xresT_t = [cpool.tile([128, E], F32, tag=f"xrT{b}", name=f"xrT{b}")
                   for b in range(2)]

        # persistent mamba-phase activations
        _mstk = ExitStack()
        mpool = _mstk.enter_context(tc.tile_pool(name="mamba", bufs=1))
        zs = [[mpool.tile([128, L], BF16, tag=f"zs{d}{b}", name=f"zs{d}{b}")
               for b in range(2)] for d in range(2)]
        xs = [[mpool.tile([128, L], BF16, tag=f"xs{d}{b}", name=f"xs{d}{b}")
               for b in range(2)] for d in range(2)]
        dt_t = [[mpool.tile([128, L], BF16, tag=f"dt{d}{b}", name=f"dt_t{d}{b}")
                 for b in range(2)] for d in range(2)]
        dtx = [[mpool.tile([128, L], BF16, tag=f"dtx{d}{b}", name=f"dtx{d}{b}")
                for b in range(2)] for d in range(2)]

        # ---------------- front half ----------------
        # dts outlives the front (dt_phase(1) runs in the scan window), so
        # it is created first (pools must be released in LIFO order).
        _dstk = ExitStack()
        dts = _dstk.enter_context(tc.tile_pool(name="dts", bufs=2))
        _fstk = ExitStack()
        p1 = _fstk.enter_context(tc.tile_pool(name="ph1", bufs=1))
        p1sq = _fstk.enter_context(tc.tile_pool(name="ph1sq", bufs=3))
        xzp = _fstk.enter_context(tc.tile_pool(name="xz", bufs=2))
        cvp = _fstk.enter_context(tc.tile_pool(name="conv", bufs=2))
        xpp = _fstk.enter_context(tc.tile_pool(name="xp", bufs=2))
        ps_st = _fstk.enter_context(
            tc.tile_pool(name="ps_st", bufs=2, space="PSUM"))
        ps_in = _fstk.enter_context(
            tc.tile_pool(name="ps_in", bufs=2, space="PSUM"))
        ps_xp = _fstk.enter_context(
            tc.tile_pool(name="ps_xp", bufs=1, space="PSUM"))
        ps_dt = _fstk.enter_context(
            tc.tile_pool(name="ps_dt", bufs=2, space="PSUM"))

        xt = [[p1.tile([128, L], BF16, tag=f"xt{k}_{b}", name=f"xt{k}_{b}")
               for b in range(2)] for k in range(8)]
        win_t = p1.tile([128, 8 * 512], BF16, tag="win")
        for k in range(8):
            nc.sync.dma_start(xt[k][0][:], xT[k * 128:(k + 1) * 128, 0:L])
        nc.sync.dma_start(win_t[:], win[:])
        nc.sync.dma_start(sw_t[:], sw_in[:])
        nc.sync.dma_start(convw_t[:], convw[:])
        nc.sync.dma_start(convb_t[:], convb[:])
        nc.sync.dma_start(xpw_t[:], xpw[:])
        for k in range(8):
            nc.sync.dma_start(xt[k][1][:], xT[k * 128:(k + 1) * 128, L:TOK])
        nc.sync.dma_start(ident_t[:], ident[:])
        nc.sync.dma_start(dtw_t[:], dtw[:])
        nc.sync.dma_start(dtb_t[:], dtb[:])
        nc.sync.dma_start(asc_t[:], a_sc[:])
        nc.sync.dma_start(dvec_t[:], dvec[:])
        if ln1b_nonzero:
            nc.sync.dma_start(sbin_t[:], sb_in[:])
        negs = [p1.tile([1, L], BF16, tag=f"negm{b}", name=f"negm{b}")
                for b in range(2)]
        r_reps = [p1.tile([128, L], BF16, tag=f"r_rep{b}", name=f"r_rep{b}")
                  for b in range(2)]
        xmp = [[xzp.tile([128, 3 + L], BF16, tag=f"xmp{d}{b}", name=f"xmp{d}{b}")
                for b in range(2)] for d in range(2)]
        for d in range(2):
            for b in range(2):
                nc.vector.memset(xmp[d][b][:, 0:3], 0.0)

        def stats(b):
            sum_sb = p1.tile([1, L], F32, tag="rows", bufs=4)
            sq_sb = p1.tile([1, L], F32, tag="rows", bufs=4)
            for ch in range(2):
                sl = slice(ch * 512, (ch + 1) * 512)
                pss = ps_st.tile([1, 512], F32, tag="pstat")
                for k in range(8):
                    nc.tensor.matmul(pss[:], ones_t[:], xt[k][b][:, sl],
                                     start=(k == 0), stop=(k == 7))
                nc.scalar.copy(sum_sb[:, sl], pss[:])
                psq = ps_st.tile([1, 512], F32, tag="pstat")
                for k in range(8):
                    xq = p1sq.tile([128, 512], BF16, tag="xq")
                    nc.vector.tensor_tensor(xq[:], xt[k][b][:, sl],
                                            xt[k][b][:, sl], OP.mult)
                    nc.tensor.matmul(psq[:], ones_t[:], xq[:],
                                     start=(k == 0), stop=(k == 7))
                nc.scalar.copy(sq_sb[:, sl], psq[:])
            m_neg = p1.tile([1, L], F32, tag="rows", bufs=4)
            nc.vector.tensor_scalar_mul(m_neg[:], sum_sb[:], -1.0 / E)
            nc.vector.tensor_copy(negs[b][:], m_neg[:])
            msq = p1.tile([1, L], F32, tag="rows", bufs=4)
            nc.vector.tensor_tensor(msq[:], m_neg[:], m_neg[:], OP.mult)
            var = p1.tile([1, L], F32, tag="rows", bufs=4)
            nc.vector.scalar_tensor_tensor(var[:], sq_sb[:], 1.0 / E,
                                           msq[:], OP.mult, OP.subtract)
            lnv = p1.tile([1, L], F32, tag="rows", bufs=4)
            nc.scalar.activation(lnv[:], var[:], AF.Ln, bias=eps_t[0:1, :])
            r_sb = p1.tile([1, L], BF16, tag="rowsb", bufs=2)
            nc.scalar.activation(r_sb[:], lnv[:], AF.Exp, scale=-0.5)
            for hh in range(2):
                prb = ps_in.tile([128, 512], F32, tag="ps")
                nc.tensor.matmul(prb[:], ones_row[:],
                                 r_sb[:, hh * 512:(hh + 1) * 512],
                                 start=True, stop=True)
                nc.scalar.copy(r_reps[b][:, hh * 512:(hh + 1) * 512], prb[:])

        def in_proj_mt(b, mt, dst_of_ch):
            """One 128-row chunk of in_proj for batch-half b."""
            for ch in range(2):
                col = ch * 512
                ps = ps_in.tile([128, 512], F32, tag="ps")
                for k in range(8):
                    nc.tensor.matmul(
                        ps[:],
                        win_t[:, k * 512 + mt * 128:k * 512 + (mt + 1) * 128],
                        xt[k][b][:, col:col + 512], start=(k == 0), stop=False)
                nc.tensor.matmul(ps[:], sw_t[:, mt * 128:(mt + 1) * 128],
                                 negs[b][:, col:col + 512],
                                 start=False, stop=True)
                dst = dst_of_ch(ch)
                if ln1b_nonzero:
                    tmp = p1sq.tile([128, 512], F32, tag="eptmp")
                    nc.vector.tensor_tensor(tmp[:], ps[:],
                                            r_reps[b][:, col:col + 512],
                                            OP.mult)
                    nc.scalar.activation(dst, tmp[:], AF.Identity,
                                         bias=sbin_t[:, mt:mt + 1])
                else:
                    nc.vector.tensor_tensor(dst, ps[:],
                                            r_reps[b][:, col:col + 512],
                                            OP.mult)

        def conv_taps(b):
            """DVE tap accumulation for conv; Silu is issued separately."""
            accs = []
            for d in range(2):
                acc0 = cvp.tile([128, L], BF16, tag="acc", bufs=4)
                nc.vector.tensor_scalar_mul(acc0[:], xmp[d][b][:, 0:L],
                                            convw_t[:, d * KC:d * KC + 1])
                for k in (1, 2, 3):
                    acc1 = cvp.tile([128, L], BF16, tag="acc", bufs=4)
                    nc.vector.scalar_tensor_tensor(
                        acc1[:], xmp[d][b][:, k:k + L],
                        convw_t[:, d * KC + k:d * KC + k + 1],
                        acc0[:], OP.mult, OP.add)
                    acc0 = acc1
                accs.append(acc0)
            return accs

        def conv_silu(b, accs):
            for d in range(2):
                nc.scalar.activation(xs[d][b][:], accs[d][:], AF.Silu,
                                     bias=convb_t[:, d:d + 1])

        def xproj(b):
            """x_proj partials + AllReduce + dtr/bc extraction."""
            xdblp = xpp.tile([96, L], BF16, tag="xdbl")
            for ch in range(2):
                col = ch * 512
                psx = ps_xp.tile([96, 512], F32, tag="psx")
                for k in range(2):
                    nc.tensor.matmul(psx[:], xpw_t[:, k * 96:(k + 1) * 96],
                                     xs[k][b][:, col:col + 512],
                                     start=(k == 0), stop=(k == 1))
                nc.scalar.copy(xdblp[:, col:col + 512], psx[:])
            nc.sync.dma_start(ar_ins[b][:], xdblp[:])
            nc.gpsimd.collective_compute("AllReduce", OP.add,
                                         ins=[ar_ins[b][:]],
                                         outs=[ar_outs[b][:]],
                                         replica_groups=RG)
            nc.gpsimd.dma_start(dtr_t[:, b * L:(b + 1) * L],
                                ar_outs[b][0:64, :])
            nc.gpsimd.dma_start(bc_bfs[b][:], ar_outs[b][64:96, :])

        dtr_t = dts.tile([64, TOK], BF16, tag="dtrb")

        def dt_phase(b, pspool):
            """dt = softplus(dt_proj + bias): batched Exps, then Lns."""
            dtes = []
            for mt in range(2):
                dte_t = dts.tile([128, L], BF16, tag="dte", bufs=2)
                for ch in range(2):
                    col = ch * 512
                    psd = pspool.tile([128, 512], F32, tag="mlp")
                    nc.tensor.matmul(psd[:],
                                     dtw_t[:, mt * 128:(mt + 1) * 128],
                                     dtr_t[:, b * L + col:b * L + col + 512],
                                     start=True, stop=True)
                    nc.scalar.activation(dte_t[:, col:col + 512],
                                         psd[:], AF.Exp,
                                         bias=dtb_t[:, mt:mt + 1])
                dtes.append(dte_t)
            for mt in range(2):
                nc.scalar.activation(dt_t[mt][b][:], dtes[mt][:], AF.Ln,
                                     bias=1.0)

        def dtx_mults(b):
            for mt in range(2):
                nc.vector.tensor_tensor(dtx[mt][b][:], dt_t[mt][b][:],
                                        xs[mt][b][:], OP.mult)

        def z_rows_mm(b):
            """PE matmuls + DVE epilogue for z; Silu issued separately."""
            zts = []
            for mt in range(2, 4):
                zt = xzp.tile([128, L], BF16, tag=f"zt{mt - 2}{b}",
                              name=f"zt{mt - 2}{b}")
                in_proj_mt(b, mt, lambda ch, zt=zt:
                           zt[:, ch * 512:ch * 512 + 512])
                zts.append(zt)
            return zts

        def z_silu(b, zts):
            for i, zt in enumerate(zts):
                nc.scalar.activation(zs[i][b][:], zt[:], AF.Silu)

        # ---- front emission order ----
        # Scalar: [lnexp: stats b0] [silu: conv b0] [lnexp: stats b1, dt b0]
        #         [silu: conv b1, z b0, z b1] [lnexp: a-exps, dt b1, ...]
        stats(0)
        for mt in range(2):
            in_proj_mt(0, mt, lambda ch, mt=mt:
                       xmp[mt][0][:, 3 + ch * 512:3 + ch * 512 + 512])
        accs0 = conv_taps(0)
        conv_silu(0, accs0)
        xproj(0)
        stats(1)
        for mt in range(2):
            in_proj_mt(1, mt, lambda ch, mt=mt:
                       xmp[mt][1][:, 3 + ch * 512:3 + ch * 512 + 512])
        accs1 = conv_taps(1)
        conv_silu(1, accs1)
        xproj(1)
        dt_phase(0, ps_dt)
        zts0 = z_rows_mm(0)
        z_silu(0, zts0)
        zts1 = z_rows_mm(1)
        z_silu(1, zts1)
        dtx_mults(0)
        _fstk.close()

        # ---- weights for the back half ----
        _wstk = ExitStack()
        wp = _wstk.enter_context(tc.tile_pool(name="wts", bufs=1, side="right"))
        wstr = _wstk.enter_context(
            tc.tile_pool(name="wstr", bufs=2, side="right"))
        wo_t = wp.tile([128, 16 * 8 * 128], BF16, tag="wo")

        # ---- scan + per-(d,b) A2A + per-b backend ----
        _sstk = ExitStack()
        pa = _sstk.enter_context(tc.tile_pool(name="scan_a", bufs=4))
        pbh = _sstk.enter_context(tc.tile_pool(name="scan_bh", bufs=5))
        pr = _sstk.enter_context(tc.tile_pool(name="scan_r", bufs=5))
        py = _sstk.enter_context(tc.tile_pool(name="scan_y", bufs=2))
        ps_y = _sstk.enter_context(
            tc.tile_pool(name="ps_y", bufs=2, space="PSUM"))
        bk = _sstk.enter_context(tc.tile_pool(name="bke", bufs=1))
        bk2 = _sstk.enter_context(tc.tile_pool(name="bke2", bufs=2))
        bk4 = _sstk.enter_context(tc.tile_pool(name="bke4", bufs=2))
        ps_po = _sstk.enter_context(
            tc.tile_pool(name="ps_po", bufs=1, space="PSUM"))
        ps_mlp = _sstk.enter_context(
            tc.tile_pool(name="ps_mlp", bufs=2, space="PSUM"))

        G = 2
        NG = NST // G

        # a-tiles: plane-1 col0 stays 0 forever; exps write plane1[1:] only.
        a_tiles = [pa.tile([128, G, L], BF16, tag="a", name=f"a_init{i}")
                   for i in range(4)]
        for t in a_tiles:
            nc.vector.memset(t[:, 1, 0:1], 0.0)

        def a_exp(d, b, g):
            """Scalar exps for group g of (d,b). Returns the a-tile."""
            a_t = pa.tile([128, G, L], BF16, tag="a")
            for j in range(G):
                n = g * G + j
                dst = a_t[:, j, :] if j == 0 else a_t[:, j, 1:L]
                src = dt_t[d][b][:] if j == 0 else dt_t[d][b][:, 1:L]
                nc.scalar.activation(
                    dst, src, AF.Exp,
                    scale=asc_t[:, d * NST + n:d * NST + n + 1])
            return a_t

        def reps(b, g, split=False):
            eng = nc.sync if b == 0 else nc.scalar
            brep = pr.tile([128, G, L], BF16, tag="bcr")
            crep = pr.tile([128, G, L], BF16, tag="bcr")
            if split:
                for j in range(G):
                    for hv in range(2):
                        cs = slice(hv * 512, (hv + 1) * 512)
                        eng.dma_start(
                            brep[:, j, cs],
                            _rep0(bc_bfs[b][g * G + j:g * G + j + 1, cs]))
                        eng.dma_start(
                            crep[:, j, cs],
                            _rep0(bc_bfs[b][16 + g * G + j:
                                            16 + g * G + j + 1, cs]))
            else:
                eng.dma_start(
                    brep[:], _rep0(bc_bfs[b][g * G:(g + 1) * G, :]))
                eng.dma_start(
                    crep[:], _rep0(bc_bfs[b][16 + g * G:16 + (g + 1) * G, :]))
            return brep, crep

        def scan_group(d, b, g, a_t, brep, crep, psy):
            bx = pbh.tile([128, G, L], BF16, tag="bxhc")
            dslice = dtx[d][b][:]
            dxb = bass.AP(dslice.tensor, dslice.offset,
                          [list(dslice.ap[0]), [0, G], [1, L]])
            nc.vector.tensor_tensor(bx[:], dxb, brep[:], OP.mult)
            h_t = pbh.tile([128, G, L], BF16, tag="bxhc")
            nc.vector.tensor_tensor_scan(
                h_t[:].rearrange("p a b -> p (a b)"),
                a_t[:].rearrange("p a b -> p (a b)"),
                bx[:].rearrange("p a b -> p (a b)"),
                0.0, OP.mult, OP.add)
            hc = pbh.tile([128, G, L], BF16, tag="bxhc")
            nc.vector.tensor_tensor(hc[:], h_t[:], crep[:], OP.mult)
            for j in range(G):
                for hh in range(2):
                    nc.tensor.matmul(
                        psy[:, hh * 512:(hh + 1) * 512], ident_t[:],
                        hc[:, j, hh * 512:(hh + 1) * 512],
                        start=(g == 0 and j == 0),
                        stop=(g == NG - 1 and j == G - 1))

        def scan_tail(d, b, psy):
            """y2/y3 + pack for one d-half; combined A2A fires at d==1."""
            y2 = py.tile([128, L], BF16, tag="y2")
            nc.vector.scalar_tensor_tensor(y2[:], xs[d][b][:],
                                           dvec_t[:, d:d + 1], psy[:],
                                           OP.mult, OP.add)
            y3 = py.tile([128, L], BF16, tag="y3")
            nc.vector.tensor_tensor(y3[:], y2[:], zs[d][b][:], OP.mult)
            nc.scalar.dma_start(
                a2a_ins[d][b][:].rearrange("c (p q) -> p c q", p=128),
                y3[:].rearrange("p (c q) -> p c q", c=NC))
            nc.gpsimd.collective_compute("AllToAll", OP.bypass,
                                         ins=[a2a_ins[d][b][:]],
                                         outs=[a2a_outs[d][b][:]],
                                         replica_groups=RG)

        def scan_b(b, work=None):
            """All 16 state-groups for both d-halves of batch-half b.

            For b==1 the d1 stream lags d0 by one group so d0's final
            A2A fires a half-group earlier (pulls the tail forward).
            work[g]: callable issued after slot g for overlap work.
            """
            lag = 1 if b == 1 else 0
            psys = [ps_y.tile([128, L], F32, tag="psy", name=f"psy{d}{b}")
                    for d in range(2)]
            # rep-DMA prefetch first (they gate the first bx), then the
            # Scalar exp run-ahead.
            pending_r = {g: reps(b, g, split=True) for g in range(2)}
            pending_a = {}
            for g in range(2):
                for d in range(2):
                    pending_a[(d, g)] = a_exp(d, b, g)
            for slot in range(NG + lag):
                for d in range(2):
                    gd = slot - d * lag
                    if not (0 <= gd < NG):
                        continue
                    brep, crep = pending_r[gd]
                    if d == 1 or lag == 0 and d == 1:
                        pass
                    a_t = pending_a.pop((d, gd))
                    scan_group(d, b, gd, a_t, brep, crep, psys[d])
                    if gd == NG - 1:
                        scan_tail(d, b, psys[d])
                    if d == 1 or (lag and gd == NG - 1 and d == 0 and False):
                        pending_r.pop(gd)
                if lag == 0 and slot in pending_r and slot + 2 < NG:
                    pass
                if slot + 2 < NG:
                    pending_r[slot + 2] = reps(b, slot + 2)
                for d in range(2):
                    ga = slot + 2 - d * lag
                    if 2 <= ga < NG and (d, ga) not in pending_a:
                        pending_a[(d, ga)] = a_exp(d, b, ga)
                if work and slot < len(work) and work[slot]:
                    work[slot]()

        def scan_b_seq(b, work_d0=None, work_d1=None):
            """Sequential-d scans: d0's A2A/out_proj overlap d1's window."""
            for d in range(2):
                psy = ps_y.tile([128, L], F32, tag="psy", name=f"psyq{d}{b}")
                pending_r = {g: reps(b, g) for g in range(2)}
                pending_a = {g: a_exp(d, b, g) for g in range(2)}
                work = work_d0 if d == 0 else work_d1
                for g in range(NG):
                    brep, crep = pending_r.pop(g)
                    a_t = pending_a.pop(g)
                    scan_group(d, b, g, a_t, brep, crep, psy)
                    if g == NG - 1:
                        scan_tail(d, b, psy)
                    if g + 2 < NG:
                        pending_r[g + 2] = reps(b, g + 2)
                        pending_a[g + 2] = a_exp(d, b, g + 2)
                    if work and g < len(work) and work[g]:
                        work[g]()

        def out_proj_half(d, b, po):
            """Accumulate the d-half contribution into po."""
            yf = bk2.tile([128, NC, TOKB], BF16, tag="yf", name=f"yf{d}{b}")
            nc.scalar.dma_start(
                yf[:], a2a_outs[d][b][:].rearrange("i (p q) -> p i q", p=128))
            for i in range(NC):
                for ech in range(2):
                    nc.tensor.matmul(
                        po[:, ech * 512:(ech + 1) * 512],
                        yf[:, i, :],
                        wo_t[:, (d * 8 + i) * 1024 + ech * 512:
                             (d * 8 + i) * 1024 + (ech + 1) * 512],
                        start=(d == 0 and i == 0),
                        stop=(d == 1 and i == NC - 1),
                        skip_group_check=True)

        def backend_pre(b, po):
            """r1_T -> per-partition LN2 -> r1n (transposed back) for half b."""
            r1_T = bk.tile([128, E], F32, tag="r1T", name=f"r1T_{b}")
            nc.vector.tensor_tensor(r1_T[:], po[:], xresT_t[b][:], OP.add)
            r1n_T = bk.tile([128, E], BF16, tag="r1nT", name=f"r1nT_{b}")
            sum2 = bk.tile([128, 1], F32, tag="sum2")
            nc.scalar.activation(r1n_T[:], r1_T[:], AF.Copy, accum_out=sum2[:])
            sq2t = bk.tile([128, 1], F32, tag="sq2")
            nc.scalar.activation(r1n_T[:], r1_T[:], AF.Square, accum_out=sq2t[:])
            m2n = bk.tile([128, 1], F32, tag="m2n")
            nc.vector.tensor_scalar_mul(m2n[:], sum2[:], -1.0 / E)
            msq2 = bk.tile([128, 1], F32, tag="msq2")
            nc.vector.tensor_tensor(msq2[:], m2n[:], m2n[:], OP.mult)
            var2 = bk.tile([128, 1], F32, tag="var2")
            nc.vector.scalar_tensor_tensor(var2[:], sq2t[:], 1.0 / E, msq2[:],
                                           OP.mult, OP.subtract)
            lnv2 = bk.tile([128, 1], F32, tag="lnv2")
            nc.scalar.activation(lnv2[:], var2[:], AF.Ln, bias=eps_t[:])
            rstd = bk.tile([128, 1], F32, tag="rstd")
            nc.scalar.activation(rstd[:], lnv2[:], AF.Exp, scale=-0.5)
            nbias = bk.tile([128, 1], F32, tag="nbias")
            nc.vector.tensor_tensor(nbias[:], m2n[:], rstd[:], OP.mult)
            nc.scalar.activation(r1n_T[:], r1_T[:], AF.Identity,
                                 bias=nbias[:], scale=rstd[:])
            r1n = [bk.tile([128, TOKB], BF16, tag=f"r1n{m}", name=f"r1n{b}_{m}")
                   for m in range(8)]
            for m in range(8):
                tp = ps_mlp.tile([128, TOKB], BF16, tag="mlp",
                                 name=f"tpr{b}_{m}")
                nc.tensor.transpose(tp[:], r1n_T[:, m * 128:(m + 1) * 128],
                                    ident_t[:])
                if b == 1:
                    nc.vector.tensor_copy(r1n[m][:], tp[:])
                else:
                    nc.scalar.copy(r1n[m][:], tp[:])
            h1 = [bk.tile([128, TOKB], BF16, tag=f"h1_{m}", name=f"h1_{b}_{m}")
                  for m in range(32)]
            h1T = bk.tile([128, HID], BF16, tag="h1T", name=f"h1T_{b}")
            return {"r1_T": r1_T, "r1n": r1n, "h1": h1, "h1T": h1T}

        def backend_fc_chunk(b, st, c):
            """fc+gelu for HID cols [c*512, (c+1)*512)."""
            wfck = wstr.tile([128, 8, 512], BF16, tag="wch",
                             name=f"wfck{b}_{c}")
            for k in range(8):
                eng = nc.scalar if (b == 1 and k % 2 == 1) else nc.sync
                eng.dma_start(
                    wfck[:, k, :],
                    wfc[:, k * HID + c * 512:k * HID + (c + 1) * 512])
            psf = ps_mlp.tile([128, 512], F32, tag="mlp", name=f"psf{b}_{c}")
            for k in range(8):
                nc.tensor.matmul(psf[:], st["r1n"][k][:], wfck[:, k, :],
                                 start=(k == 0), stop=False)
            nc.tensor.matmul(psf[:], ones_row[:],
                             sbfc_sb[0:1, c * 512:(c + 1) * 512],
                             start=False, stop=True)
            nc.scalar.activation(st["h1T"][:, c * 512:(c + 1) * 512], psf[:],
                                 AF.Gelu)

        def backend_proj(b, st):
            """transpose h1T, proj (weights moving), +bias +residual, store."""
            for m in range(32):
                tp = ps_mlp.tile([128, TOKB], BF16, tag="mlp",
                                 name=f"tph{b}_{m}")
                nc.tensor.transpose(tp[:], st["h1T"][:, m * 128:(m + 1) * 128],
                                    ident_t[:])
                nc.vector.tensor_copy(st["h1"][m][:], tp[:])
            psps = [ps_mlp.tile([128, 512], F32, tag="mlp",
                                name=f"psp{b}_{h}") for h in range(2)]
            for c in range(8):
                wpjk = wstr.tile([128, 4, 1024], BF16, tag="wch",
                                 name=f"wpjk{b}_{c}")
                for kl in range(4):
                    k = c * 4 + kl
                    for hv in range(2):
                        eng = nc.scalar if (b == 1 and hv == 1) else nc.sync
                        eng.dma_start(
                            wpjk[:, kl, hv * 512:(hv + 1) * 512],
                            wpj[:, k * E + hv * 512:k * E + (hv + 1) * 512])
                for kl in range(4):
                    k = c * 4 + kl
                    for ech in range(2):
                        nc.tensor.matmul(
                            psps[ech][:], st["h1"][k][:],
                            wpjk[:, kl, ech * 512:(ech + 1) * 512],
                            start=(k == 0), stop=False,
                            skip_group_check=True)
            for ech in range(2):
                nc.tensor.matmul(psps[ech][:], ones_row[:],
                                 pjb_sb[0:1, ech * 512:(ech + 1) * 512],
                                 start=False, stop=True,
                                 skip_group_check=True)
                ot = bk4.tile([128, 512], F32, tag="ot", bufs=1)
                nc.vector.tensor_tensor(
                    ot[:], psps[ech][:],
                    st["r1_T"][:, ech * 512:(ech + 1) * 512], OP.add)
                nc.sync.dma_start(
                    outT[b * TOKB:(b + 1) * TOKB, ech * 512:(ech + 1) * 512],
                    ot[:])

        # ---- scan phase emission ----
        # b0 scans: dt/dtx for b1 + weight/residual loads injected.
        def b0_g0():
            for w8 in range(8):
                nc.sync.dma_start(wo_t[:, w8 * 2048:(w8 + 1) * 2048],
                                  wo[:, w8 * 2048:(w8 + 1) * 2048])
            dt_phase(1, ps_mlp)
        def b0_g1():
            dtx_mults(1)
        def b0_g2():
            for b2 in range(2):
                nc.sync.dma_start(xresT_t[b2][:],
                                  xresT[b2 * TOKB:(b2 + 1) * TOKB, :])
            nc.sync.dma_start(sbfc_sb[:], sbfc[:])
            nc.sync.dma_start(pjb_sb[:], pjb[:])
        scan_b(0, work=[b0_g0, b0_g1, b0_g2, None, None, None, None, None])

        # b1 scans with backend(b0) injected.
        po0 = ps_po.tile([128, E], F32, tag="po", name="po0")
        st_holder = {}
        def b1_g0():
            out_proj_half(0, 0, po0)
            out_proj_half(1, 0, po0)
        def b1_g1():
            st_holder["st0"] = backend_pre(0, po0)
        def b1_fc(c0):
            def f():
                for c in range(c0, c0 + 2):
                    backend_fc_chunk(0, st_holder["st0"], c)
            return f
        def b1_g2():
            for c in range(4):
                backend_fc_chunk(0, st_holder["st0"], c)
        scan_b(1, work=[b1_g0, b1_g1, b1_g2, None, None, None, None, None])

        # rest of backend(b0) emitted here: covers the b1 A2A wait on PE
        for c in range(4, 8):
            backend_fc_chunk(0, st_holder["st0"], c)
        backend_proj(0, st_holder["st0"])

        # tail: backend(b1)
        po1 = ps_po.tile([128, E], F32, tag="po", name="po1")
        out_proj_half(0, 1, po1)
        out_proj_half(1, 1, po1)
        st1 = backend_pre(1, po1)
        for c in range(8):
            backend_fc_chunk(1, st1, c)
        backend_proj(1, st1)
        _sstk.close()
        _wstk.close()
        _dstk.close()
        _mstk.close()

    nc.compile()
    _BUILD_CACHE[key] = nc
    return nc


def _prep_inputs(inputs):
    """Host-side sharding/packing. Returns list of per-core input dicts."""
    f32 = np.float32
    x = np.asarray(inputs["x"], f32)
    ln1_w = np.asarray(inputs["ln1_w"], f32)
    ln1_b = np.asarray(inputs["ln1_b"], f32)
    in_proj_w = np.asarray(inputs["in_proj_w"], f32)
    conv_w = np.asarray(inputs["conv_w"], f32)
    conv_b = np.asarray(inputs["conv_b"], f32)
    x_proj_w = np.asarray(inputs["x_proj_w"], f32)
    dt_proj_w = np.asarray(inputs["dt_proj_w"], f32)
    dt_proj_b = np.asarray(inputs["dt_proj_b"], f32)
    A_log = np.asarray(inputs["A_log"], f32)
    D = np.asarray(inputs["D"], f32)
    out_proj_w = np.asarray(inputs["out_proj_w"], f32)
    ln2_w = np.asarray(inputs["ln2_w"], f32)
    ln2_b = np.asarray(inputs["ln2_b"], f32)
    fc_w = np.asarray(inputs["fc_w"], f32)
    fc_b = np.asarray(inputs["fc_b"], f32)
    proj_w = np.asarray(inputs["proj_w"], f32)
    proj_b = np.asarray(inputs["proj_b"], f32)

    xT_f = np.ascontiguousarray(x.reshape(TOK, E).T)          # [E, TOK]
    xT_b = xT_f.astype(bf)

    def pack_lhsT(lhsT, dt=bf):
        K, M = lhsT.shape
        nk = K // 128
        return np.ascontiguousarray(
            lhsT.reshape(nk, 128, M).transpose(1, 0, 2).reshape(128, nk * M)
        ).astype(dt)

    Wp = in_proj_w * ln1_w[None, :]
    sb_full = in_proj_w @ ln1_b
    ln1b_nonzero = bool(np.any(sb_full != 0.0))

    Wfc = fc_w * ln2_w[None, :]
    sbfc_full = fc_w @ ln2_b + fc_b
    sbfc_row = np.ascontiguousarray(sbfc_full[None, :]).astype(bf)
    pjb_row = np.ascontiguousarray(proj_b[None, :]).astype(bf)
    wfc_pack = pack_lhsT(np.ascontiguousarray(Wfc.T))
    wpj_pack = pack_lhsT(np.ascontiguousarray(proj_w.T))
    woT = np.ascontiguousarray(out_proj_w.T)                   # [DIN, E]
    wo_blocks = []
    for d in range(2):
        for i in range(NC):
            ch0 = i * DL + d * 128
            blk = woT[ch0:ch0 + 128, :]                        # [128, E]
            wo_blocks.append(blk)
    wo_pack = np.ascontiguousarray(np.concatenate(wo_blocks, axis=1)).astype(bf)
    x_f = x.reshape(TOK, E)

    A = -np.exp(A_log)

    per_core = []
    for c in range(NC):
        dsl = slice(c * DL, (c + 1) * DL)
        rows = np.concatenate([Wp[dsl], Wp[DIN + c * DL:DIN + (c + 1) * DL]])
        win_pack = pack_lhsT(np.ascontiguousarray(rows.T))
        sw_row = rows.sum(1)[None, :].astype(bf)
        sb_rows = np.concatenate([sb_full[dsl],
                                  sb_full[DIN + c * DL:DIN + (c + 1) * DL]])
        sb_pack = np.ascontiguousarray(sb_rows.reshape(4, 128).T).astype(f32)

        cw = conv_w[dsl, 0, :]
        convw_pack = np.ascontiguousarray(
            cw.reshape(2, 128, KC).transpose(1, 0, 2).reshape(128, 2 * KC)
        ).astype(f32)
        convb_pack = np.ascontiguousarray(
            conv_b[dsl].reshape(2, 128).T).astype(f32)

        xpw_pack = pack_lhsT(np.ascontiguousarray(x_proj_w[:, dsl].T))
        dtw_slice = np.ascontiguousarray(dt_proj_w[dsl].T).astype(bf)
        dtb_pack = np.ascontiguousarray(
            dt_proj_b[dsl].reshape(2, 128).T).astype(f32)
        asc_pack = np.ascontiguousarray(
            A[dsl].reshape(2, 128, NST).transpose(1, 0, 2).reshape(128, 2 * NST)
        ).astype(f32)
        dvec_pack = np.ascontiguousarray(D[dsl].reshape(2, 128).T).astype(f32)

        xresT_slice = np.ascontiguousarray(np.concatenate(
            [x_f[c * TOKB:(c + 1) * TOKB, :],
             x_f[L + c * TOKB:L + (c + 1) * TOKB, :]], axis=0))

        per_core.append({
            "xT": xT_b, "win": win_pack, "sw_in": sw_row, "sb_in": sb_pack,
            "convw": convw_pack, "convb": convb_pack, "xpw": xpw_pack,
            "dtw": dtw_slice, "dtb": dtb_pack, "a_sc": asc_pack,
            "dvec": dvec_pack, "wo": wo_pack, "xresT": xresT_slice,
            "wfc": wfc_pack, "sbfc": sbfc_row,
            "wpj": wpj_pack, "pjb": pjb_row,
            "ones128": np.ones((128, 1), bf),
            "ident": np.eye(128, dtype=bf),
        })
    return per_core, ln1b_nonzero


def kernel(**inputs):
    per_core, ln1b_nonzero = _prep_inputs(inputs)
    nc = _build(ln1b_nonzero)
    trace = bool(int(os.environ.get("BASSK_TRACE", "0")))
    try:
        res = run_bass_kernel_spmd(nc, per_core, core_ids=list(range(NC)),
                                   trace=trace)
    except Exception:
        # transient device hiccups (e.g. NRT exec-unit errors) clear on retry
        res = run_bass_kernel_spmd(nc, per_core, core_ids=list(range(NC)),
                                   trace=trace)
    kernel.last_results = res
    out_full = np.empty((TOK, E), np.float32)
    for c in range(NC):
        o = res.results[c]["outT"]
        out_full[c * TOKB:(c + 1) * TOKB] = o[:TOKB]
        out_full[L + c * TOKB:L + (c + 1) * TOKB] = o[TOKB:]
    return out_full.reshape(B, L, E).astype(np.float32)
